# revision 1
# baseline (speedup 1.0000x reference)
"""Trainium2 Bass kernel for nn_Conv2dMem (bit-slice fake-quantized 3x3 conv).

Math (per image): unfold 3x3/pad1 -> per-row granule (32 along K, C-major)
symmetric int7 fake-quant of activations; per 32x32 block fake-quant of
weights; GEMM; bias.

Strategy (8 cores, batch-parallel, 1 image/core):
  - Weights fake-quantized exactly on host (numpy), fed as fp16 GEMM tiles.
  - Image stays in (C x H*W) layout; conv = 18 shifted GEMM accumulations
    (2 channel-tiles x 9 kernel positions) into PSUM, n on partitions.
  - Granule absmax m[g, l] built on-device from image-domain max algebra
    (A=|x|, H2/H3 horizontal, V2/V3 vertical window maxes) + partition-strided
    gather DMAs exploiting the LCM(32 channels, 9 granules) periodicity.
  - Scales r = 63/m, s = m/63 broadcast granule->channel rows via tiny 0/1
    matmuls on the tensor engine (K=36 stationary E).
  - Quantize: t = x*r (DVE), q = round(t) via +C-C magic (exact RNE),
    xdq = q*s -> fp16 -> GEMM moving operand.
"""
import os
import numpy as np
from contextlib import ExitStack

C_IN = 256
N_OUT = 256
H = W = 56
HP = WP = 58
L = H * W            # 3136
KS = 3
GRAN = 32
NG = (C_IN * KS * KS) // GRAN   # 72 granules
MAXQ = 63.0
RND_C = 3.0 * 2 ** 22           # magic rounding constant (fp32 RNE to integer)
NCT = 2                          # channel partition tiles (256/128)
NH = 2                           # output-channel halves
CHUNK = 448                      # l-chunk (8 rows of 56); 7 chunks; fits PSUM bank
NCHUNK = L // CHUNK
ROWS = CHUNK // W                # 8
M_CLAMP = 6e-5                   # fp16-safe clamp for zero-granule guard

ARR_NAMES = ("A", "H2", "H3", "V2", "V3")


# --------------------------------------------------------------------------
# host-side index tables
# --------------------------------------------------------------------------
def granule_terms():
    """For each g' in [0,9): list of (c'', arr, da, db) whose pointwise max
    over terms equals the granule absmax. Padded to 6 terms (repeats)."""
    out = []
    for gp in range(9):
        c_lo, c_hi = (32 * gp) // 9, (32 * gp + 31) // 9
        j0 = 32 * gp - 9 * c_lo
        j1 = 32 * gp + 32 - 9 * c_hi
        terms = []
        if j0 == 0:
            terms.append((c_lo, "V3", 0, 0))
        else:
            dh0, dw0 = divmod(j0, 3)
            terms.append((c_lo, {0: "H3", 1: "H2", 2: "A"}[dw0], dh0, dw0))
            if dh0 == 0:
                terms.append((c_lo, "V2", 1, 0))
            elif dh0 == 1:
                terms.append((c_lo, "H3", 2, 0))
        for c in range(c_lo + 1, c_hi):
            terms.append((c, "V3", 0, 0))
        if j1 == 9:
            terms.append((c_hi, "V3", 0, 0))
        else:
            q, rr = divmod(j1, 3)
            if q == 1:
                terms.append((c_hi, "H3", 0, 0))
            elif q == 2:
                terms.append((c_hi, "V2", 0, 0))
            if rr == 1:
                terms.append((c_hi, "A", q, 0))
            elif rr == 2:
                terms.append((c_hi, "H2", q, 0))
        assert 1 <= len(terms) <= 6, (gp, terms)
        while len(terms) < 6:
            terms.append(terms[0])
        out.append(terms)
    return out


TERMS = granule_terms()


def build_E():
    """E[idx= ct*9+j] : (36, 128) fp16; RJ = E.T @ r[36ct:36ct+36]."""
    E = np.zeros((NCT * 9, 36, 128), np.float16)
    for ct in range(NCT):
        for j in range(9):
            for p in range(128):
                c = 128 * ct + p
                g = (9 * c + j) // 32
                E[ct * 9 + j, g - 36 * ct, p] = 1.0
    return E


def quantize_weight_host(weight):
    """Exact numpy replica of reference _fake_quant_weight on w2d=(K,N)."""
    w2d = weight.reshape(N_OUT, -1).T.astype(np.float32)      # (2304, 256)
    K, N = w2d.shape
    wg = w2d.reshape(K // 32, 32, N // 32, 32)
    max_abs = np.max(np.abs(wg), axis=(1, 3), keepdims=True)
    scale = (max_abs / np.float32(MAXQ)).astype(np.float32)
    scale = np.where(scale == 0, np.float32(1.0), scale)
    q = np.clip(np.round(wg / scale), -MAXQ, MAXQ)
    deq = (q * scale).astype(np.float32).reshape(K, N)
    return deq


def pack_weights(wdq):
    """(2304, 256) -> W[idx=ct*9+j, nh, p, n] fp16 stationary tiles."""
    Wt = np.zeros((NCT * 9, NH, 128, 128), np.float16)
    for ct in range(NCT):
        for j in range(9):
            rows = (9 * (128 * ct + np.arange(128)) + j)      # (128,)
            for nh in range(NH):
                Wt[ct * 9 + j, nh] = wdq[rows][:, 128 * nh:128 * nh + 128]
    return Wt


def pad_image(x):
    """(256,56,56) fp32 -> (2,128,3364) fp16 padded."""
    xp = np.pad(x, ((0, 0), (1, 1), (1, 1))).astype(np.float16)
    return xp.reshape(NCT, 128, HP * WP)


# --------------------------------------------------------------------------
# numpy model of the device pipeline (for validation in test.py)
# --------------------------------------------------------------------------
def model_core(x, Wt, bias):
    """Bit-accurate-ish numpy model of what the bass kernel computes for one
    image. x: (256,56,56) fp32. Returns (256,56,56) fp32."""
    xp16 = pad_image(x)                                      # (2,128,3364) f16
    xp = xp16.reshape(C_IN, HP, WP)
    A = np.abs(xp)
    H2 = np.maximum(A[:, :, :-1], A[:, :, 1:])               # (C,58,57)
    H3 = np.maximum(H2[:, :, :-1], A[:, :, 2:])              # (C,58,56)
    V2 = np.maximum(H3[:, :-1], H3[:, 1:])                   # (C,57,56)
    V3 = np.maximum(V2[:, :-1], H3[:, 2:])                   # (C,56,56)
    arrs = {"A": A, "H2": H2, "H3": H3, "V2": V2, "V3": V3}

    m = np.zeros((NG, H, W), np.float16)
    for g in range(NG):
        B, gp = divmod(g, 9)
        acc = None
        for (cpp, arr, da, db) in TERMS[gp]:
            c = 32 * B + cpp
            v = arrs[arr][c, da:da + H, db:db + W]
            acc = v if acc is None else np.maximum(acc, v)
        m[g] = acc
    m = np.maximum(m, np.float16(M_CLAMP))

    m32 = m.astype(np.float32)
    rj = (np.float32(MAXQ) * (1.0 / m32)).astype(np.float16).astype(np.float32)
    sj = (m.astype(np.float32) * np.float32(1.0 / MAXQ)).astype(np.float16).astype(np.float32)

    out = np.zeros((N_OUT, L), np.float32)
    x16 = xp16.reshape(C_IN, HP, WP).astype(np.float32)
    for ct in range(NCT):
        for j in range(9):
            dh, dw = divmod(j, 3)
            cs = np.arange(128 * ct, 128 * ct + 128)
            g = (9 * cs + j) // 32                            # (128,)
            xv = x16[cs, dh:dh + H, dw:dw + W].reshape(128, L)
            t = xv * rj[g].reshape(128, L)
            q = (t + RND_C) - RND_C                           # fp32 RNE round
            xdq = (q.astype(np.float32) * sj[g].reshape(128, L)).astype(np.float16)
            for nh in range(NH):
                Wtile = Wt[ct * 9 + j, nh].astype(np.float32)  # (128c,128n)
                out[128 * nh:128 * nh + 128] += Wtile.T @ xdq.astype(np.float32)
    out += bias.astype(np.float32)[:, None]
    return out.reshape(N_OUT, H, W)


# --------------------------------------------------------------------------
# bass kernel
# --------------------------------------------------------------------------
_CACHE = {}


def _build_nc():
    import concourse.bass as bass
    import concourse.bacc as bacc
    import concourse.mybir as mybir
    from concourse import tile

    f32, f16 = mybir.dt.float32, mybir.dt.float16
    ALU = mybir.AluOpType
    ACTF = mybir.ActivationFunctionType

    nc = bacc.Bacc("TRN2", target_bir_lowering=False, debug=False)
    xpad_d = nc.dram_tensor("xpad", (NCT, 128, HP * WP), f16, kind="ExternalInput")
    w_d = nc.dram_tensor("wt", (128, NCT * 9 * NH * 128), f16, kind="ExternalInput")
    e_d = nc.dram_tensor("et", (36, NCT * 9 * 128), f16, kind="ExternalInput")
    b_d = nc.dram_tensor("bias", (128, NH), f32, kind="ExternalInput")
    y_d = nc.dram_tensor("y", (NH, 128, L), f32, kind="ExternalOutput")

    es = ExitStack()
    with tile.TileContext(nc) as tc:
        pc = es.enter_context(tc.tile_pool(name="consts", bufs=1))
        pst = es.enter_context(tc.tile_pool(name="staging", bufs=1))
        pw = es.enter_context(tc.tile_pool(name="work", bufs=8))
        pyo = es.enter_context(tc.tile_pool(name="yout", bufs=2))
        prj = es.enter_context(tc.tile_pool(name="rps", bufs=2, space="PSUM"))
        psj = es.enter_context(tc.tile_pool(name="sps", bufs=2, space="PSUM"))
        py0 = es.enter_context(tc.tile_pool(name="yps0", bufs=1, space="PSUM"))
        py1 = es.enter_context(tc.tile_pool(name="yps1", bufs=1, space="PSUM"))
        pa = tc.alloc_tile_pool(name="arrays", bufs=1)
        pT = tc.alloc_tile_pool(name="Tstage", bufs=1)
        psc = tc.alloc_tile_pool(name="scaletmp", bufs=1)

        # ---- load constants -------------------------------------------------
        xp_sb = [pc.tile([128, HP * WP], f16, tag=f"xp{ct}", name=f"xp{ct}") for ct in range(NCT)]
        for ct in range(NCT):
            nc.sync.dma_start(out=xp_sb[ct][:], in_=xpad_d.ap()[ct])
        w_sb = pc.tile([128, NCT * 9 * NH * 128], f16, tag="wsb")
        nc.sync.dma_start(out=w_sb[:], in_=w_d.ap())
        e_sb = pc.tile([36, NCT * 9 * 128], f16, tag="esb")
        nc.sync.dma_start(out=e_sb[:], in_=e_d.ap())
        bias_sb = pc.tile([128, NH], f32, tag="bsb")
        nc.sync.dma_start(out=bias_sb[:], in_=b_d.ap())

        # ---- stage 1: image-domain max algebra (fp16) -----------------------
        arrs = {}
        for name in ARR_NAMES:
            arrs[name] = [pa.tile([128, HP * WP], f16, tag=f"{name}{ct}", name=f"{name}{ct}")
                          for ct in range(NCT)]
        NPW = HP * WP  # 3364
        for ct in range(NCT):
            A, H2a, H3a, V2a, V3a = (arrs[n][ct] for n in ARR_NAMES)
            nc.scalar.activation(A[:], xp_sb[ct][:], ACTF.Abs)
            nc.vector.tensor_tensor(H2a[:, 0:NPW - 1], A[:, 0:NPW - 1],
                                    A[:, 1:NPW], op=ALU.max)
            nc.vector.tensor_tensor(H3a[:, 0:NPW - 2], H2a[:, 0:NPW - 2],
                                    A[:, 2:NPW], op=ALU.max)
            nc.vector.tensor_tensor(V2a[:, 0:NPW - HP], H3a[:, 0:NPW - HP],
                                    H3a[:, HP:NPW], op=ALU.max)
            nc.vector.tensor_tensor(V3a[:, 0:NPW - 2 * HP], V2a[:, 0:NPW - 2 * HP],
                                    H3a[:, 2 * HP:NPW], op=ALU.max)

        # ---- stage 2: granule gather + max tree -----------------------------
        m16 = pst.tile([NG, L], f16, tag="m16")
        T_prev = None
        for i in range(6):
            T_i = pT.tile([NG, L], f16, tag="T", name=f"T{i}", bufs=3)
            for gp in range(9):
                cpp, arr, da, db = TERMS[gp][i]
                for half in range(NCT):
                    src = (arrs[arr][half]
                           .rearrange("p (a b) -> p a b", a=HP)
                           [cpp:cpp + 97:32, da:da + H, db:db + W])
                    dst = (T_i
                           .rearrange("p (a b) -> p a b", a=H)
                           [36 * half + gp:36 * half + gp + 28:9])
                    nc.sync.dma_start(out=dst, in_=src)
            if i == 1:
                nc.vector.tensor_tensor(m16[:], T_prev[:], T_i[:], op=ALU.max)
            elif i > 1:
                nc.vector.tensor_tensor(m16[:], m16[:], T_i[:], op=ALU.max)
            T_prev = T_i
        nc.vector.tensor_scalar(m16[:], m16[:], float(M_CLAMP), None, op0=ALU.max)

        # ---- stage 3: scales ------------------------------------------------
        m32 = psc.tile([NG, L], f32, tag="m32")
        rcp = psc.tile([NG, L], f32, tag="rcp")
        nc.vector.tensor_copy(m32[:], m16[:])
        nc.vector.reciprocal_approx_fast(out=rcp[:], in_=m32[:])
        rj = pst.tile([NG, L], f16, tag="rj")
        sj = pst.tile([NG, L], f16, tag="sj")
        nc.vector.tensor_scalar(rj[:], rcp[:], float(MAXQ), None, op0=ALU.mult)
        nc.vector.tensor_scalar(sj[:], m16[:], float(1.0 / MAXQ), None, op0=ALU.mult)
        r1 = pst.tile([36, L], f16, tag="r1")
        s1 = pst.tile([36, L], f16, tag="s1")
        nc.sync.dma_start(out=r1[:], in_=rj[36:72, :])
        nc.sync.dma_start(out=s1[:], in_=sj[36:72, :])
        psc.release()
        pT.release()
        pa.release()
        r_src = [rj[0:36], r1[:]]
        s_src = [sj[0:36], s1[:]]

        # ---- stage 4: main loop --------------------------------------------
        for ch in range(NCHUNK):
            h0 = ROWS * ch
            lsl = slice(CHUNK * ch, CHUNK * (ch + 1))
            yps = [py0.tile([128, CHUNK], f32, tag="y0", name="y0"),
                   py1.tile([128, CHUNK], f32, tag="y1", name="y1")]
            for idx in range(NCT * 9):
                ct, j = divmod(idx, 9)
                dh, dw = divmod(j, 3)
                rj_ps = prj.tile([128, CHUNK], f32, tag="rjps")
                sj_ps = psj.tile([128, CHUNK], f32, tag="sjps")
                e_ap = e_sb[:, 128 * idx:128 * idx + 128]
                nc.tensor.matmul(rj_ps[:], e_ap, r_src[ct][:, lsl],
                                 start=True, stop=True)
                nc.tensor.matmul(sj_ps[:], e_ap, s_src[ct][:, lsl],
                                 start=True, stop=True)
                xv = (xp_sb[ct].rearrange("p (a b) -> p a b", a=HP)
                      [:, h0 + dh:h0 + dh + ROWS, dw:dw + W])
                t = pw.tile([128, ROWS, W], f32, tag="t")
                q = pw.tile([128, ROWS, W], f16, tag="q")
                xdq = pw.tile([128, ROWS, W], f16, tag="xdq")
                rj3 = rj_ps.rearrange("p (a b) -> p a b", a=ROWS)
                sj3 = sj_ps.rearrange("p (a b) -> p a b", a=ROWS)
                nc.vector.tensor_tensor(t[:], xv, rj3, op=ALU.mult)
                nc.gpsimd.tensor_scalar(q[:], t[:], RND_C, -RND_C,
                                        op0=ALU.add, op1=ALU.add)
                nc.vector.tensor_tensor(xdq[:], q[:], sj3, op=ALU.mult)
                xdq_f = xdq.rearrange("p a b -> p (a b)")
                for nh in range(NH):
                    wsl = w_sb[:, (idx * NH + nh) * 128:(idx * NH + nh + 1) * 128]
                    nc.tensor.matmul(yps[nh][:], wsl, xdq_f,
                                     start=(idx == 0), stop=(idx == NCT * 9 - 1))
            for nh in range(NH):
                ysb = pyo.tile([128, CHUNK], f32, tag=f"ysb{nh}")
                nc.scalar.activation(ysb[:], yps[nh][:], ACTF.Identity,
                                     bias=bias_sb[:, nh:nh + 1], scale=1.0)
                nc.sync.dma_start(out=y_d.ap()[nh, :, lsl], in_=ysb[:])
        es.close()
    nc.compile()
    return nc


def kernel(input, weight, bias):
    input = np.asarray(input, np.float32)
    weight = np.asarray(weight, np.float32)
    bias = np.asarray(bias, np.float32)
    B = input.shape[0]
    assert B == 8 and input.shape[1:] == (C_IN, H, W)

    from concourse import bass_utils

    if "nc" not in _CACHE:
        _CACHE["nc"] = _build_nc()
    nc = _CACHE["nc"]

    wdq = quantize_weight_host(weight)
    Wt = np.ascontiguousarray(np.transpose(pack_weights(wdq), (2, 0, 1, 3))
                              ).reshape(128, NCT * 9 * NH * 128)
    E = np.ascontiguousarray(np.transpose(build_E(), (1, 0, 2))).reshape(36, NCT * 9 * 128)
    b = np.ascontiguousarray(bias.reshape(NH, 128).T).astype(np.float32)

    in_maps = []
    for bi in range(B):
        in_maps.append({
            "xpad": pad_image(input[bi]),
            "wt": Wt,
            "et": E,
            "bias": b,
        })
    res = bass_utils.run_bass_kernel_spmd(nc, in_maps, core_ids=list(range(B)))
    out = np.stack([r["y"].reshape(N_OUT, H, W) for r in res.results])
    return out.astype(np.float32)


if __name__ == "__main__":
    pass



# revision 2
# speedup vs baseline: 2.9742x; 2.9742x over previous
"""Trainium2 Bass kernel for nn_Conv2dMem (bit-slice fake-quantized 3x3 conv).

Math (per image): unfold 3x3/pad1 -> per-row granule (32 along K, C-major)
symmetric int7 fake-quant of activations; per 32x32 block fake-quant of
weights; GEMM; bias.

Strategy (8 cores, batch-parallel, 1 image/core):
  - Weights fake-quantized exactly on host (numpy), fed as fp16 GEMM tiles.
  - Image stays in (C x H*W) layout; conv = 18 shifted GEMM accumulations
    (2 channel-tiles x 9 kernel positions) into PSUM, n on partitions.
  - Granule absmax m[g, l] built on-device from image-domain max algebra
    (A=|x|, H2/H3 horizontal, V2/V3 vertical window maxes) + partition-strided
    gather DMAs exploiting the LCM(32 channels, 9 granules) periodicity.
  - Scales r = 63/m, s = m/63 broadcast granule->channel rows via tiny 0/1
    matmuls on the tensor engine (K=36 stationary E).
  - Quantize: t = x*r (DVE), q = round(t) via +C-C magic (exact RNE),
    xdq = q*s -> fp16 -> GEMM moving operand.
"""
import os
import numpy as np
from contextlib import ExitStack

C_IN = 256
N_OUT = 256
H = W = 56
HP = WP = 58
L = H * W            # 3136
KS = 3
GRAN = 32
NG = (C_IN * KS * KS) // GRAN   # 72 granules
MAXQ = 63.0
RND_C = 3.0 * 2 ** 22           # magic rounding constant (fp32 RNE to integer)
NCT = 2                          # channel partition tiles (256/128)
NH = 2                           # output-channel halves
CHUNK = 448                      # l-chunk (8 rows of 56); 7 chunks; fits PSUM bank
NCHUNK = L // CHUNK
ROWS = CHUNK // W                # 8
M_CLAMP = 6e-5                   # fp16-safe clamp for zero-granule guard

ARR_NAMES = ("A", "H2", "H3", "V2", "V3")


# --------------------------------------------------------------------------
# host-side index tables
# --------------------------------------------------------------------------
def granule_terms():
    """For each g' in [0,9): list of (c'', arr, da, db) whose pointwise max
    over terms equals the granule absmax. Padded to 6 terms (repeats)."""
    out = []
    for gp in range(9):
        c_lo, c_hi = (32 * gp) // 9, (32 * gp + 31) // 9
        j0 = 32 * gp - 9 * c_lo
        j1 = 32 * gp + 32 - 9 * c_hi
        terms = []
        if j0 == 0:
            terms.append((c_lo, "V3", 0, 0))
        else:
            dh0, dw0 = divmod(j0, 3)
            terms.append((c_lo, {0: "H3", 1: "H2", 2: "A"}[dw0], dh0, dw0))
            if dh0 == 0:
                terms.append((c_lo, "V2", 1, 0))
            elif dh0 == 1:
                terms.append((c_lo, "H3", 2, 0))
        for c in range(c_lo + 1, c_hi):
            terms.append((c, "V3", 0, 0))
        if j1 == 9:
            terms.append((c_hi, "V3", 0, 0))
        else:
            q, rr = divmod(j1, 3)
            if q == 1:
                terms.append((c_hi, "H3", 0, 0))
            elif q == 2:
                terms.append((c_hi, "V2", 0, 0))
            if rr == 1:
                terms.append((c_hi, "A", q, 0))
            elif rr == 2:
                terms.append((c_hi, "H2", q, 0))
        assert 1 <= len(terms) <= 6, (gp, terms)
        while len(terms) < 6:
            terms.append(terms[0])
        out.append(terms)
    return out


TERMS = granule_terms()


def build_E():
    """E[idx= ct*9+j] : (36, 128) fp16; RJ = E.T @ r[36ct:36ct+36]."""
    E = np.zeros((NCT * 9, 36, 128), np.float16)
    for ct in range(NCT):
        for j in range(9):
            for p in range(128):
                c = 128 * ct + p
                g = (9 * c + j) // 32
                E[ct * 9 + j, g - 36 * ct, p] = 1.0
    return E


def quantize_weight_host(weight):
    """Exact numpy replica of reference _fake_quant_weight on w2d=(K,N)."""
    w2d = weight.reshape(N_OUT, -1).T.astype(np.float32)      # (2304, 256)
    K, N = w2d.shape
    wg = w2d.reshape(K // 32, 32, N // 32, 32)
    max_abs = np.max(np.abs(wg), axis=(1, 3), keepdims=True)
    scale = (max_abs / np.float32(MAXQ)).astype(np.float32)
    scale = np.where(scale == 0, np.float32(1.0), scale)
    q = np.clip(np.round(wg / scale), -MAXQ, MAXQ)
    deq = (q * scale).astype(np.float32).reshape(K, N)
    return deq


def pack_weights(wdq):
    """(2304, 256) -> W[idx=ct*9+j, nh, p, n] fp16 stationary tiles."""
    Wt = np.zeros((NCT * 9, NH, 128, 128), np.float16)
    for ct in range(NCT):
        for j in range(9):
            rows = (9 * (128 * ct + np.arange(128)) + j)      # (128,)
            for nh in range(NH):
                Wt[ct * 9 + j, nh] = wdq[rows][:, 128 * nh:128 * nh + 128]
    return Wt


def pad_image(x):
    """(256,56,56) fp32 -> (2,128,3364) fp16 padded."""
    xp = np.pad(x, ((0, 0), (1, 1), (1, 1))).astype(np.float16)
    return xp.reshape(NCT, 128, HP * WP)


# --------------------------------------------------------------------------
# numpy model of the device pipeline (for validation in test.py)
# --------------------------------------------------------------------------
def model_core(x, Wt, bias):
    """Bit-accurate-ish numpy model of what the bass kernel computes for one
    image. x: (256,56,56) fp32. Returns (256,56,56) fp32."""
    xp16 = pad_image(x)                                      # (2,128,3364) f16
    xp = xp16.reshape(C_IN, HP, WP)
    A = np.abs(xp)
    H2 = np.maximum(A[:, :, :-1], A[:, :, 1:])               # (C,58,57)
    H3 = np.maximum(H2[:, :, :-1], A[:, :, 2:])              # (C,58,56)
    V2 = np.maximum(H3[:, :-1], H3[:, 1:])                   # (C,57,56)
    V3 = np.maximum(V2[:, :-1], H3[:, 2:])                   # (C,56,56)
    arrs = {"A": A, "H2": H2, "H3": H3, "V2": V2, "V3": V3}

    m = np.zeros((NG, H, W), np.float16)
    for g in range(NG):
        B, gp = divmod(g, 9)
        acc = None
        for (cpp, arr, da, db) in TERMS[gp]:
            c = 32 * B + cpp
            v = arrs[arr][c, da:da + H, db:db + W]
            acc = v if acc is None else np.maximum(acc, v)
        m[g] = acc
    m = np.maximum(m, np.float16(M_CLAMP))

    m32 = m.astype(np.float32)
    rj = (np.float32(MAXQ) * (1.0 / m32)).astype(np.float16).astype(np.float32)
    sj = (m.astype(np.float32) * np.float32(1.0 / MAXQ)).astype(np.float16).astype(np.float32)

    out = np.zeros((N_OUT, L), np.float32)
    x16 = xp16.reshape(C_IN, HP, WP).astype(np.float32)
    for ct in range(NCT):
        for j in range(9):
            dh, dw = divmod(j, 3)
            cs = np.arange(128 * ct, 128 * ct + 128)
            g = (9 * cs + j) // 32                            # (128,)
            xv = x16[cs, dh:dh + H, dw:dw + W].reshape(128, L)
            t = xv * rj[g].reshape(128, L)
            q = (t + RND_C) - RND_C                           # fp32 RNE round
            xdq = (q.astype(np.float32) * sj[g].reshape(128, L)).astype(np.float16)
            for nh in range(NH):
                Wtile = Wt[ct * 9 + j, nh].astype(np.float32)  # (128c,128n)
                out[128 * nh:128 * nh + 128] += Wtile.T @ xdq.astype(np.float32)
    out += bias.astype(np.float32)[:, None]
    return out.reshape(N_OUT, H, W)


# --------------------------------------------------------------------------
# bass kernel
# --------------------------------------------------------------------------
_CACHE = {}


def _build_nc():
    import concourse.bass as bass
    import concourse.bacc as bacc
    import concourse.mybir as mybir
    from concourse import tile

    f32, f16 = mybir.dt.float32, mybir.dt.float16
    ALU = mybir.AluOpType
    ACTF = mybir.ActivationFunctionType

    nc = bacc.Bacc("TRN2", target_bir_lowering=False, debug=False)
    xpad_d = nc.dram_tensor("xpad", (NCT, 128, HP * WP), f16, kind="ExternalInput")
    w_d = nc.dram_tensor("wt", (128, NCT * 9 * NH * 128), f16, kind="ExternalInput")
    e_d = nc.dram_tensor("et", (36, NCT * 9 * 128), f16, kind="ExternalInput")
    b_d = nc.dram_tensor("bias", (128, NH), f32, kind="ExternalInput")
    y_d = nc.dram_tensor("y", (NH, 128, L), f32, kind="ExternalOutput")

    es = ExitStack()
    with tile.TileContext(nc) as tc:
        pc = es.enter_context(tc.tile_pool(name="consts", bufs=1))
        pst = es.enter_context(tc.tile_pool(name="staging", bufs=1))
        pw = es.enter_context(tc.tile_pool(name="work", bufs=8))
        pyo = es.enter_context(tc.tile_pool(name="yout", bufs=2))
        prj = es.enter_context(tc.tile_pool(name="rps", bufs=2, space="PSUM"))
        psj = es.enter_context(tc.tile_pool(name="sps", bufs=2, space="PSUM"))
        py0 = es.enter_context(tc.tile_pool(name="yps0", bufs=1, space="PSUM"))
        py1 = es.enter_context(tc.tile_pool(name="yps1", bufs=1, space="PSUM"))
        pa = tc.alloc_tile_pool(name="arrays", bufs=1)
        pT = tc.alloc_tile_pool(name="Tstage", bufs=1)
        psc = tc.alloc_tile_pool(name="scaletmp", bufs=1)

        # ---- load constants -------------------------------------------------
        xp_sb = [pc.tile([128, HP * WP], f16, tag=f"xp{ct}", name=f"xp{ct}") for ct in range(NCT)]
        for ct in range(NCT):
            nc.sync.dma_start(out=xp_sb[ct][:], in_=xpad_d.ap()[ct])
        w_sb = pc.tile([128, NCT * 9 * NH * 128], f16, tag="wsb")
        nc.sync.dma_start(out=w_sb[:], in_=w_d.ap())
        e_sb = pc.tile([36, NCT * 9 * 128], f16, tag="esb")
        nc.sync.dma_start(out=e_sb[:], in_=e_d.ap())
        bias_sb = pc.tile([128, NH], f32, tag="bsb")
        nc.sync.dma_start(out=bias_sb[:], in_=b_d.ap())

        # ---- stage 1: image-domain max algebra (fp16) -----------------------
        arrs = {}
        for name in ARR_NAMES:
            arrs[name] = [pa.tile([128, HP * WP], f16, tag=f"{name}{ct}", name=f"{name}{ct}")
                          for ct in range(NCT)]
        NPW = HP * WP  # 3364
        for ct in range(NCT):
            A, H2a, H3a, V2a, V3a = (arrs[n][ct] for n in ARR_NAMES)
            nc.scalar.activation(A[:], xp_sb[ct][:], ACTF.Abs)
            nc.vector.tensor_tensor(H2a[:, 0:NPW - 1], A[:, 0:NPW - 1],
                                    A[:, 1:NPW], op=ALU.max)
            nc.vector.tensor_tensor(H3a[:, 0:NPW - 2], H2a[:, 0:NPW - 2],
                                    A[:, 2:NPW], op=ALU.max)
            nc.vector.tensor_tensor(V2a[:, 0:NPW - HP], H3a[:, 0:NPW - HP],
                                    H3a[:, HP:NPW], op=ALU.max)
            nc.vector.tensor_tensor(V3a[:, 0:NPW - 2 * HP], V2a[:, 0:NPW - 2 * HP],
                                    H3a[:, 2 * HP:NPW], op=ALU.max)

        # ---- stage 2: granule gather + max tree -----------------------------
        m16 = pst.tile([NG, L], f16, tag="m16")
        T_prev = None
        for i in range(6):
            T_i = pT.tile([NG, L], f16, tag="T", name=f"T{i}", bufs=3)
            for gp in range(9):
                cpp, arr, da, db = TERMS[gp][i]
                for half in range(NCT):
                    src = (arrs[arr][half]
                           .rearrange("p (a b) -> p a b", a=HP)
                           [cpp:cpp + 97:32, da:da + H, db:db + W])
                    dst = (T_i
                           .rearrange("p (a b) -> p a b", a=H)
                           [36 * half + gp:36 * half + gp + 28:9])
                    nc.sync.dma_start(out=dst, in_=src)
            if i == 1:
                nc.vector.tensor_tensor(m16[:], T_prev[:], T_i[:], op=ALU.max)
            elif i > 1:
                nc.vector.tensor_tensor(m16[:], m16[:], T_i[:], op=ALU.max)
            T_prev = T_i
        nc.vector.tensor_scalar(m16[:], m16[:], float(M_CLAMP), None, op0=ALU.max)

        # ---- stage 3: scales ------------------------------------------------
        m32 = psc.tile([NG, L], f32, tag="m32")
        rcp = psc.tile([NG, L], f32, tag="rcp")
        nc.vector.tensor_copy(m32[:], m16[:])
        nc.vector.reciprocal_approx_fast(out=rcp[:], in_=m32[:])
        rj = pst.tile([NG, L], f16, tag="rj")
        sj = pst.tile([NG, L], f16, tag="sj")
        nc.vector.tensor_scalar(rj[:], rcp[:], float(MAXQ), None, op0=ALU.mult)
        nc.vector.tensor_scalar(sj[:], m16[:], float(1.0 / MAXQ), None, op0=ALU.mult)
        r1 = pst.tile([36, L], f16, tag="r1")
        s1 = pst.tile([36, L], f16, tag="s1")
        nc.sync.dma_start(out=r1[:], in_=rj[36:72, :])
        nc.sync.dma_start(out=s1[:], in_=sj[36:72, :])
        psc.release()
        pT.release()
        pa.release()
        r_src = [rj[0:36], r1[:]]
        s_src = [sj[0:36], s1[:]]

        # ---- stage 4: main loop --------------------------------------------
        for ch in range(NCHUNK):
            h0 = ROWS * ch
            lsl = slice(CHUNK * ch, CHUNK * (ch + 1))
            yps = [py0.tile([128, CHUNK], f32, tag="y0", name="y0"),
                   py1.tile([128, CHUNK], f32, tag="y1", name="y1")]
            for idx in range(NCT * 9):
                ct, j = divmod(idx, 9)
                dh, dw = divmod(j, 3)
                rj_ps = prj.tile([128, CHUNK], f32, tag="rjps")
                sj_ps = psj.tile([128, CHUNK], f32, tag="sjps")
                e_ap = e_sb[:, 128 * idx:128 * idx + 128]
                nc.tensor.matmul(rj_ps[:], e_ap, r_src[ct][:, lsl],
                                 start=True, stop=True)
                nc.tensor.matmul(sj_ps[:], e_ap, s_src[ct][:, lsl],
                                 start=True, stop=True)
                xv = (xp_sb[ct].rearrange("p (a b) -> p a b", a=HP)
                      [:, h0 + dh:h0 + dh + ROWS, dw:dw + W])
                t = pw.tile([128, ROWS, W], f32, tag="t")
                q = pw.tile([128, ROWS, W], f16, tag="q")
                xdq = pw.tile([128, ROWS, W], f16, tag="xdq")
                rj3 = rj_ps.rearrange("p (a b) -> p a b", a=ROWS)
                sj3 = sj_ps.rearrange("p (a b) -> p a b", a=ROWS)
                nc.vector.tensor_tensor(t[:], xv, rj3, op=ALU.mult)
                nc.vector.tensor_scalar(q[:], t[:], RND_C, -RND_C,
                                        op0=ALU.add, op1=ALU.add)
                nc.vector.tensor_tensor(xdq[:], q[:], sj3, op=ALU.mult)
                xdq_f = xdq.rearrange("p a b -> p (a b)")
                for nh in range(NH):
                    wsl = w_sb[:, (idx * NH + nh) * 128:(idx * NH + nh + 1) * 128]
                    nc.tensor.matmul(yps[nh][:], wsl, xdq_f,
                                     start=(idx == 0), stop=(idx == NCT * 9 - 1))
            for nh in range(NH):
                ysb = pyo.tile([128, CHUNK], f32, tag=f"ysb{nh}")
                nc.scalar.activation(ysb[:], yps[nh][:], ACTF.Identity,
                                     bias=bias_sb[:, nh:nh + 1], scale=1.0)
                nc.sync.dma_start(out=y_d.ap()[nh, :, lsl], in_=ysb[:])
        es.close()
    nc.compile()
    return nc


def kernel(input, weight, bias):
    input = np.asarray(input, np.float32)
    weight = np.asarray(weight, np.float32)
    bias = np.asarray(bias, np.float32)
    B = input.shape[0]
    assert B == 8 and input.shape[1:] == (C_IN, H, W)

    from concourse import bass_utils

    if "nc" not in _CACHE:
        _CACHE["nc"] = _build_nc()
    nc = _CACHE["nc"]

    wdq = quantize_weight_host(weight)
    Wt = np.ascontiguousarray(np.transpose(pack_weights(wdq), (2, 0, 1, 3))
                              ).reshape(128, NCT * 9 * NH * 128)
    E = np.ascontiguousarray(np.transpose(build_E(), (1, 0, 2))).reshape(36, NCT * 9 * 128)
    b = np.ascontiguousarray(bias.reshape(NH, 128).T).astype(np.float32)

    in_maps = []
    for bi in range(B):
        in_maps.append({
            "xpad": pad_image(input[bi]),
            "wt": Wt,
            "et": E,
            "bias": b,
        })
    res = bass_utils.run_bass_kernel_spmd(nc, in_maps, core_ids=list(range(B)))
    out = np.stack([r["y"].reshape(N_OUT, H, W) for r in res.results])
    return out.astype(np.float32)


if __name__ == "__main__":
    pass

